# revision 12
# baseline (speedup 1.0000x reference)
"""Trainium2 Bass kernel for a 2-layer GCN discriminator (GCNConv -> sigmoid) x2.

Strategy
--------
With N=4096 nodes and E=262144 edges (avg degree 64), the gather/segment-sum
message passing is densified: the normalized adjacency
A[d, s] = sum_{edges (s,d)} dinv[s]*dinv[d]  (+ dinv[i]^2 self loops)
is built on the host as a dense 4096x4096 matrix.  The whole network is then

    x1  = sigmoid(A @ (x @ W1) + b1)
    out = sigmoid(A @ (x1 @ W2) + b2)

i.e. two dense 4096^3 GEMMs plus small epilogues -- ideal for the PE array.

Sharding over 8 cores: layer-1 is column-sharded (each core computes 512
columns of H = x@W1 and of x1), the tiny layer-2 contraction
h2 = x1 @ W2 is computed per-core on the local 512 columns and summed across
cores with a 16KB AllGather + on-core tree sum (split in two chunks so the
first mostly overlaps GEMM2), and the final out rows are row-sharded
(512 rows per core).

The two big GEMMs run in fp8-e4m3 with DoubleRow perf mode (2 contraction
rows per PE cell per cycle).  W1 and A are pre-scaled by 64 on the host so
their entries sit in e4m3's normal range; the scale is removed for free in
the activation epilogues (ACT computes func(in*scale + bias)).  PSUM
accumulation is fp32 throughout; the small final GEMM stays bf16.
"""

import numpy as np
import ml_dtypes

N = 4096
E = 262144
P = 128
NCORES = 8
JC = N // NCORES          # 512 feature-cols (layer1) / out-rows (final) per core
KT = N // P               # 32 contraction tiles
MT = N // P               # 32 output row tiles
MSPLIT = 16               # collective chunk A covers m-tiles [0, MSPLIT)
WSCALE = 64.0             # host pre-scale on W1 and A for fp8 range

_BF16 = ml_dtypes.bfloat16
_FP8 = ml_dtypes.float8_e4m3

_CACHE = {}


def _build_bass_program():
    """Build + compile the SPMD Bass program (identical on all 8 cores)."""
    import concourse.bass as bass  # noqa: F401
    import concourse.bacc as bacc
    import concourse.tile as tile
    import concourse.mybir as mybir
    from concourse.bass_interp import get_hw_module

    dt = mybir.dt
    AF = mybir.ActivationFunctionType
    DR = mybir.MatmulPerfMode.DoubleRow

    nc = bacc.Bacc("TRN2", target_bir_lowering=False, debug=False,
                   num_devices=NCORES)

    # ---- kernel I/O (per-core) ----
    # xp_t[m, p, t*128+c] = x[m*128+c, t*128+p]      (pre-tiled lhsT panels)
    xp_t = nc.dram_tensor("xp_t", [MT, P, N], dt.float8e4, kind="ExternalInput")
    # ap_t[m, p, t*128+c] = 64*AT[t*128+p, m*128+c]  (AT[s,d] = A[d,s])
    ap_t = nc.dram_tensor("ap_t", [MT, P, N], dt.float8e4, kind="ExternalInput")
    # w1_t[p, t*512+j] = 64*W1[t*128+p, c*JC+j]
    w1_t = nc.dram_tensor("w1_t", [P, KT * JC], dt.float8e4, kind="ExternalInput")
    # atrc_t[p, t*512+j] = AT[t*128+p, c*JC+j]       (final-stage rhs panels)
    atrc_t = nc.dram_tensor("atrc_t", [P, KT * JC], dt.bfloat16, kind="ExternalInput")
    b1c = nc.dram_tensor("b1c", [1, JC], dt.bfloat16, kind="ExternalInput")  # 64*b1
    w2b = nc.dram_tensor("w2b", [P, JC], dt.float32, kind="ExternalInput")
    b2v = nc.dram_tensor("b2v", [1, 1], dt.float32, kind="ExternalInput")
    outc = nc.dram_tensor("outc", [1, JC], dt.float32, kind="ExternalOutput")

    MA = MSPLIT
    MB = MT - MSPLIT

    with tile.TileContext(nc) as tc:
        with tc.tile_pool(name="const", bufs=1) as const, \
             tc.tile_pool(name="xpool", bufs=6) as xpool, \
             tc.tile_pool(name="apool", bufs=6) as apool, \
             tc.tile_pool(name="x1pool", bufs=3) as x1pool, \
             tc.tile_pool(name="pspool", bufs=2, space="PSUM") as pspool, \
             tc.tile_pool(name="drampool", bufs=1, space="DRAM") as drampool:

            # ---- resident SBUF tensors ----
            w1_sb = const.tile([P, KT, JC], dt.float8e4)
            atrc_sb = const.tile([P, KT * JC], dt.bfloat16)
            h_sb = const.tile([P, MT, JC], dt.float8e4)
            w2_sb = const.tile([P, JC], dt.float32)
            b1_sb = const.tile([1, JC], dt.bfloat16)
            b2_sb = const.tile([1, 1], dt.float32)
            ones_sb = const.tile([1, P], dt.bfloat16)
            p2_sb = const.tile([P, MT], dt.float32)
            h2fa_sb = const.tile([P, MSPLIT], dt.float32)
            h2ba_sb = const.tile([P, MSPLIT], dt.bfloat16)
            h2fb_sb = const.tile([P, MT - MSPLIT], dt.float32)
            h2bb_sb = const.tile([P, MT - MSPLIT], dt.bfloat16)
            ga_sb = const.tile([P, NCORES, MSPLIT], dt.float32)
            gb_sb = const.tile([P, NCORES, MT - MSPLIT], dt.float32)
            o_sb = const.tile([1, JC], dt.float32)
            tts_sb = const.tile([P, JC], dt.float32)

            # First matmul needs xp[0] + the first w1 chunk; keep those at the
            # head of the DMA queue (each dma_start has ~1us first-byte cost,
            # so w1 goes in 4 chunks, not 32 small ones).
            xp0 = xpool.tile([P, KT, P], dt.float8e4, tag="xp")
            nc.sync.dma_start(
                w1_sb[:, 0:2, :],
                w1_t.ap()[:, 0:2 * JC].rearrange("p (t j) -> p t j", j=JC))
            nc.sync.dma_start(xp0[:], xp_t.ap()[0].rearrange("p (t c) -> p t c", c=P))
            W1CH = 6
            for k in range(2, KT, W1CH):
                ke = min(k + W1CH, KT)
                nc.sync.dma_start(
                    w1_sb[:, k:ke, :],
                    w1_t.ap()[:, k * JC:ke * JC].rearrange(
                        "p (t j) -> p t j", j=JC))
            nc.gpsimd.dma_start(w2_sb[:], w2b.ap())
            nc.gpsimd.dma_start(b1_sb[:], b1c.ap())
            nc.gpsimd.dma_start(b2_sb[:], b2v.ap())
            nc.vector.memset(ones_sb[:], 1.0)

            # ---- GEMM 1 (fp8 DoubleRow): 64*H[:, Cc] = x @ (64*W1[:, Cc]) ----
            for m in range(MT):
                if m == 0:
                    xp = xp0
                else:
                    xp = xpool.tile([P, KT, P], dt.float8e4, tag="xp")
                    nc.sync.dma_start(
                        xp[:], xp_t.ap()[m].rearrange("p (t c) -> p t c", c=P))
                ps1 = pspool.tile([P, JC], dt.float32, tag="ps1", bufs=3)
                for k in range(0, KT, 2):
                    nc.tensor.matmul(
                        ps1[:],
                        xp[:, k:k + 2, :],
                        w1_sb[:, k:k + 2, :],
                        start=(k == 0),
                        stop=(k == KT - 2),
                        perf_mode=DR,
                    )
                # PSUM -> SBUF: H = (64H)/64, cast to fp8 (rhs of GEMM 2)
                nc.scalar.mul(h_sb[:, m, :], ps1[:], 1.0 / WSCALE)


            # ---- GEMM 2 (fp8 DoubleRow): 64*O1 = (64*A) @ H + 64*b1 ;
            #      x1 = sigmoid(O1) ; partial2[i] = sum_j x1[i,j] * W2[Cc_j] ----
            p2a_dram = drampool.tile([P, MA], dt.float32)
            p2b_dram = drampool.tile([P, MB], dt.float32)
            g2a_dram = drampool.tile([NCORES * P, MA], dt.float32, addr_space="Shared")
            g2b_dram = drampool.tile([NCORES * P, MB], dt.float32, addr_space="Shared")

            ATCH = (KT * JC) // 4
            for m in range(MT):
                app = apool.tile([P, KT, P], dt.float8e4, tag="app")
                nc.sync.dma_start(
                    app[:], ap_t.ap()[m].rearrange("p (t c) -> p t c", c=P))
                if 1 <= m <= 4:
                    # final-stage rhs: 4 x ~1MB chunks slipped between the
                    # early ap panels (small, absorbable bubbles on the queue)
                    a0 = (m - 1) * ATCH
                    nc.sync.dma_start(atrc_sb[:, a0:a0 + ATCH],
                                      atrc_t.ap()[:, a0:a0 + ATCH])
                ps2 = pspool.tile([P, JC], dt.float32, tag="ps2", bufs=3)
                for k in range(0, KT, 2):
                    nc.tensor.matmul(
                        ps2[:],
                        app[:, k:k + 2, :],
                        h_sb[:, k:k + 2, :],
                        start=(k == 0),
                        stop=False,
                        perf_mode=DR,
                    )
                # + 64*b1 as a bf16 rank-1 update: ones[1,128].T @ (64*b1)[1,512]
                nc.tensor.matmul(ps2[:], ones_sb[:], b1_sb[:], start=False, stop=True)
                x1t = x1pool.tile([P, JC], dt.float32, tag="x1t")
                # sigmoid((64*O1 + 64*b1) / 64)
                nc.scalar.activation(x1t[:], ps2[:], AF.Sigmoid, scale=1.0 / WSCALE)
                nc.vector.tensor_tensor(out=tts_sb[:], in0=x1t[:], in1=w2_sb[:],
                                        op=mybir.AluOpType.mult)
                nc.vector.tensor_reduce(out=p2_sb[:, m:m + 1], in_=tts_sb[:],
                                        axis=mybir.AxisListType.X,
                                        op=mybir.AluOpType.add)
                if m == MSPLIT - 1:
                    # chunk A: AllGather (half the ring steps of AllReduce) +
                    # on-core DVE sum; overlaps the rest of GEMM2
                    nc.gpsimd.dma_start(p2a_dram[:], p2_sb[:, 0:MA])
                    nc.gpsimd.collective_compute(
                        "AllGather", mybir.AluOpType.bypass,
                        replica_groups=[list(range(NCORES))],
                        ins=[p2a_dram.opt()], outs=[g2a_dram.opt()])
                    nc.gpsimd.dma_start(
                        ga_sb[:], g2a_dram.rearrange("(r p) m -> p r m", p=P))
                    nc.vector.tensor_tensor(out=ga_sb[:, 0, :], in0=ga_sb[:, 0, :],
                                            in1=ga_sb[:, 1, :],
                                            op=mybir.AluOpType.add)
                    nc.vector.tensor_tensor(out=ga_sb[:, 2, :], in0=ga_sb[:, 2, :],
                                            in1=ga_sb[:, 3, :],
                                            op=mybir.AluOpType.add)
                    nc.vector.tensor_tensor(out=ga_sb[:, 4, :], in0=ga_sb[:, 4, :],
                                            in1=ga_sb[:, 5, :],
                                            op=mybir.AluOpType.add)
                    nc.vector.tensor_tensor(out=ga_sb[:, 6, :], in0=ga_sb[:, 6, :],
                                            in1=ga_sb[:, 7, :],
                                            op=mybir.AluOpType.add)
                    nc.vector.tensor_tensor(out=ga_sb[:, 0, :], in0=ga_sb[:, 0, :],
                                            in1=ga_sb[:, 2, :],
                                            op=mybir.AluOpType.add)
                    nc.vector.tensor_tensor(out=ga_sb[:, 4, :], in0=ga_sb[:, 4, :],
                                            in1=ga_sb[:, 6, :],
                                            op=mybir.AluOpType.add)
                    nc.vector.tensor_tensor(out=h2fa_sb[:], in0=ga_sb[:, 0, :],
                                            in1=ga_sb[:, 4, :],
                                            op=mybir.AluOpType.add)
                    nc.vector.tensor_copy(out=h2ba_sb[:], in_=h2fa_sb[:])

            nc.gpsimd.dma_start(p2b_dram[:], p2_sb[:, MA:MT])
            nc.gpsimd.collective_compute(
                "AllGather", mybir.AluOpType.bypass,
                replica_groups=[list(range(NCORES))],
                ins=[p2b_dram.opt()], outs=[g2b_dram.opt()])
            nc.gpsimd.dma_start(
                gb_sb[:], g2b_dram.rearrange("(r p) m -> p r m", p=P))
            nc.vector.tensor_tensor(out=gb_sb[:, 0, :], in0=gb_sb[:, 0, :],
                                    in1=gb_sb[:, 1, :], op=mybir.AluOpType.add)
            nc.vector.tensor_tensor(out=gb_sb[:, 2, :], in0=gb_sb[:, 2, :],
                                    in1=gb_sb[:, 3, :], op=mybir.AluOpType.add)
            nc.vector.tensor_tensor(out=gb_sb[:, 4, :], in0=gb_sb[:, 4, :],
                                    in1=gb_sb[:, 5, :], op=mybir.AluOpType.add)
            nc.vector.tensor_tensor(out=gb_sb[:, 6, :], in0=gb_sb[:, 6, :],
                                    in1=gb_sb[:, 7, :], op=mybir.AluOpType.add)
            nc.vector.tensor_tensor(out=gb_sb[:, 0, :], in0=gb_sb[:, 0, :],
                                    in1=gb_sb[:, 2, :], op=mybir.AluOpType.add)
            nc.vector.tensor_tensor(out=gb_sb[:, 4, :], in0=gb_sb[:, 4, :],
                                    in1=gb_sb[:, 6, :], op=mybir.AluOpType.add)
            nc.vector.tensor_tensor(out=h2fb_sb[:], in0=gb_sb[:, 0, :],
                                    in1=gb_sb[:, 4, :], op=mybir.AluOpType.add)
            nc.vector.tensor_copy(out=h2bb_sb[:], in_=h2fb_sb[:])

            # ---- final (bf16): out[Rc]^T = sigmoid(h2^T @ AT[:, Rc] + b2) ----
            # k < MA uses chunk A (available while chunk B's AllReduce runs)
            ps3 = pspool.tile([1, JC], dt.float32, tag="ps3", bufs=1)
            for k in range(KT):
                lhs = h2ba_sb[:, k:k + 1] if k < MA else h2bb_sb[:, k - MA:k - MA + 1]
                nc.tensor.matmul(
                    ps3[:],
                    lhs,
                    atrc_sb[:, k * JC:(k + 1) * JC],
                    start=(k == 0),
                    stop=(k == KT - 1),
                )
            nc.scalar.activation(o_sb[:], ps3[:], AF.Sigmoid, bias=b2_sb[:])
            nc.gpsimd.dma_start(outc.ap(), o_sb[:])

    nc.compile()
    nc.m = get_hw_module(nc.m)
    return nc


def _host_preprocess(x, edge_index, W1, b1, W2, b2):
    """Build dense AT + pre-tiled fp8/bf16 operands; returns per-core in_maps."""
    edge_index = np.asarray(edge_index)
    src = edge_index[0].astype(np.int64)
    dst = edge_index[1].astype(np.int64)
    deg = np.bincount(dst, minlength=N).astype(np.float64) + 1.0
    dinv = 1.0 / np.sqrt(deg)
    vals = dinv[src] * dinv[dst]
    # AT[s, d] = A[d, s] (accumulates duplicate edges, like segment_sum)
    AT = np.bincount(src * N + dst, weights=vals, minlength=N * N)
    AT = AT.reshape(N, N)
    idx = np.arange(N)
    AT[idx, idx] += dinv * dinv
    AT32 = AT.astype(np.float32)

    x32 = np.asarray(x, dtype=np.float32)
    W1_32 = np.asarray(W1, dtype=np.float32)
    b1_32 = np.asarray(b1, dtype=np.float32)
    W2_32 = np.asarray(W2, dtype=np.float32).reshape(N)
    b2_32 = np.asarray(b2, dtype=np.float32).reshape(1)

    # xp_t[m, p, t*128+c] = x[m*128+c, t*128+p]
    xp_t = np.ascontiguousarray(
        x32.reshape(MT, P, KT, P).transpose(0, 3, 2, 1).reshape(MT, P, N)
    ).astype(_FP8)
    # ap_t[m, p, t*128+c] = 64*AT[t*128+p, m*128+c]
    ap_t = np.ascontiguousarray(
        (AT32 * np.float32(WSCALE)).reshape(KT, P, MT, P)
        .transpose(2, 1, 0, 3).reshape(MT, P, N)
    ).astype(_FP8)

    AT_b = AT32.astype(_BF16)
    W1_s = (W1_32 * np.float32(WSCALE)).astype(_FP8)

    in_maps = []
    for c in range(NCORES):
        cols = slice(c * JC, (c + 1) * JC)
        w1_t = np.ascontiguousarray(
            W1_s[:, cols].reshape(KT, P, JC).transpose(1, 0, 2).reshape(P, KT * JC)
        )
        atrc_t = np.ascontiguousarray(
            AT_b[:, cols].reshape(KT, P, JC).transpose(1, 0, 2).reshape(P, KT * JC)
        )
        in_maps.append({
            "xp_t": xp_t,
            "ap_t": ap_t,
            "w1_t": w1_t,
            "atrc_t": atrc_t,
            "b1c": (b1_32[cols] * np.float32(WSCALE)).reshape(1, JC).astype(_BF16),
            "w2b": np.ascontiguousarray(
                np.broadcast_to(W2_32[cols][None, :], (P, JC))
            ).astype(np.float32),
            "b2v": b2_32.reshape(1, 1).astype(np.float32),
        })
    return in_maps


def kernel(x, edge_index, W1, b1, W2, b2, _trace=False, _premaps=None):
    from concourse import bass_utils

    if "nc" not in _CACHE:
        _CACHE["nc"] = _build_bass_program()
    nc = _CACHE["nc"]

    in_maps = _premaps if _premaps is not None else _host_preprocess(
        x, edge_index, W1, b1, W2, b2)

    res = bass_utils.run_bass_kernel_spmd(
        nc, in_maps, core_ids=list(range(NCORES)), trace=_trace,
    )
    out = np.concatenate(
        [np.asarray(res.results[c]["outc"]).reshape(JC) for c in range(NCORES)]
    ).reshape(N, 1).astype(np.float32)
    if _trace:
        _CACHE["last_result"] = res
    return out


# revision 13
# speedup vs baseline: 1.0416x; 1.0416x over previous
"""Trainium2 Bass kernel for a 2-layer GCN discriminator (GCNConv -> sigmoid) x2.

Strategy
--------
With N=4096 nodes and E=262144 edges (avg degree 64), the gather/segment-sum
message passing is densified: the normalized adjacency
A[d, s] = sum_{edges (s,d)} dinv[s]*dinv[d]  (+ dinv[i]^2 self loops)
is built on the host as a dense 4096x4096 matrix.  The whole network is then

    x1  = sigmoid(A @ (x @ W1) + b1)
    out = sigmoid(A @ (x1 @ W2) + b2)

i.e. two dense 4096^3 GEMMs plus small epilogues -- ideal for the PE array.

Sharding over 8 cores: layer-1 is column-sharded (each core computes 512
columns of H = x@W1 and of x1), the tiny layer-2 contraction
h2 = x1 @ W2 is computed per-core on the local 512 columns and summed across
cores with a 16KB AllGather + on-core tree sum (split in two chunks so the
first mostly overlaps GEMM2), and the final out rows are row-sharded
(512 rows per core).

The two big GEMMs run in fp8-e4m3 with DoubleRow perf mode (2 contraction
rows per PE cell per cycle).  W1 and A are pre-scaled by 64 on the host so
their entries sit in e4m3's normal range; the scale is removed for free in
the activation epilogues (ACT computes func(in*scale + bias)).  PSUM
accumulation is fp32 throughout; the small final GEMM stays bf16.
"""

import numpy as np
import ml_dtypes

N = 4096
E = 262144
P = 128
NCORES = 8
JC = N // NCORES          # 512 feature-cols (layer1) / out-rows (final) per core
KT = N // P               # 32 contraction tiles
MT = N // P               # 32 output row tiles
MSPLIT = 16               # collective chunk A covers m-tiles [0, MSPLIT)
WSCALE = 64.0             # host pre-scale on W1 and A for fp8 range

_BF16 = ml_dtypes.bfloat16
_FP8 = ml_dtypes.float8_e4m3

_CACHE = {}


def _build_bass_program():
    """Build + compile the SPMD Bass program (identical on all 8 cores)."""
    import concourse.bass as bass  # noqa: F401
    import concourse.bacc as bacc
    import concourse.tile as tile
    import concourse.mybir as mybir
    from concourse.bass_interp import get_hw_module

    dt = mybir.dt
    AF = mybir.ActivationFunctionType
    DR = mybir.MatmulPerfMode.DoubleRow

    nc = bacc.Bacc("TRN2", target_bir_lowering=False, debug=False,
                   num_devices=NCORES)

    # ---- kernel I/O (per-core) ----
    # xp_t[m, p, t*128+c] = x[m*128+c, t*128+p]      (pre-tiled lhsT panels)
    xp_t = nc.dram_tensor("xp_t", [MT, P, N], dt.float8e4, kind="ExternalInput")
    # ap_t[m, p, t*128+c] = 64*AT[t*128+p, m*128+c]  (AT[s,d] = A[d,s])
    ap_t = nc.dram_tensor("ap_t", [MT, P, N], dt.float8e4, kind="ExternalInput")
    # w1_t[p, t*512+j] = 64*W1[t*128+p, c*JC+j]
    w1_t = nc.dram_tensor("w1_t", [P, KT * JC], dt.float8e4, kind="ExternalInput")
    # atrc_t[p, t*512+j] = AT[t*128+p, c*JC+j]       (final-stage rhs panels)
    atrc_t = nc.dram_tensor("atrc_t", [P, KT * JC], dt.bfloat16, kind="ExternalInput")
    b1c = nc.dram_tensor("b1c", [1, JC], dt.bfloat16, kind="ExternalInput")  # 64*b1
    w2b = nc.dram_tensor("w2b", [P, JC], dt.float32, kind="ExternalInput")
    b2v = nc.dram_tensor("b2v", [1, 1], dt.float32, kind="ExternalInput")
    outc = nc.dram_tensor("outc", [1, JC], dt.float32, kind="ExternalOutput")

    MA = MSPLIT
    MB = MT - MSPLIT

    with tile.TileContext(nc) as tc:
        with tc.tile_pool(name="const", bufs=1) as const, \
             tc.tile_pool(name="xpool", bufs=4) as xpool, \
             tc.tile_pool(name="apool", bufs=4) as apool, \
             tc.tile_pool(name="x1pool", bufs=3) as x1pool, \
             tc.tile_pool(name="pspool", bufs=2, space="PSUM") as pspool, \
             tc.tile_pool(name="drampool", bufs=1, space="DRAM") as drampool:

            # ---- resident SBUF tensors ----
            w1_sb = const.tile([P, KT, JC], dt.float8e4)
            atrc_sb = const.tile([P, KT * JC], dt.bfloat16)
            h_sb = const.tile([P, MT, JC], dt.float8e4)
            w2_sb = const.tile([P, JC], dt.float32)
            b1_sb = const.tile([1, JC], dt.bfloat16)
            b2_sb = const.tile([1, 1], dt.float32)
            ones_sb = const.tile([1, P], dt.bfloat16)
            p2_sb = const.tile([P, MT], dt.float32)
            h2fa_sb = const.tile([P, MSPLIT], dt.float32)
            h2ba_sb = const.tile([P, MSPLIT], dt.bfloat16)
            h2fb_sb = const.tile([P, MT - MSPLIT], dt.float32)
            h2bb_sb = const.tile([P, MT - MSPLIT], dt.bfloat16)
            ga_sb = const.tile([P, NCORES, MSPLIT], dt.float32)
            gb_sb = const.tile([P, NCORES, MT - MSPLIT], dt.float32)
            o_sb = const.tile([1, JC], dt.float32)
            tts_sb = const.tile([P, JC], dt.float32)

            # First matmul needs xp[0] + the first w1 chunk; keep those at the
            # head of the DMA queue (each dma_start has ~1us first-byte cost,
            # so w1 goes in 4 chunks, not 32 small ones).
            xp0 = xpool.tile([P, KT, P], dt.float8e4, tag="xp")
            nc.sync.dma_start(
                w1_sb[:, 0:2, :],
                w1_t.ap()[:, 0:2 * JC].rearrange("p (t j) -> p t j", j=JC))
            nc.sync.dma_start(xp0[:], xp_t.ap()[0].rearrange("p (t c) -> p t c", c=P))
            W1CH = 10
            for k in range(2, KT, W1CH):
                ke = min(k + W1CH, KT)
                nc.sync.dma_start(
                    w1_sb[:, k:ke, :],
                    w1_t.ap()[:, k * JC:ke * JC].rearrange(
                        "p (t j) -> p t j", j=JC))
            nc.gpsimd.dma_start(w2_sb[:], w2b.ap())
            nc.gpsimd.dma_start(b1_sb[:], b1c.ap())
            nc.gpsimd.dma_start(b2_sb[:], b2v.ap())
            nc.vector.memset(ones_sb[:], 1.0)

            # ---- GEMM 1 (fp8 DoubleRow): 64*H[:, Cc] = x @ (64*W1[:, Cc]) ----
            for m in range(MT):
                if m == 0:
                    xp = xp0
                else:
                    xp = xpool.tile([P, KT, P], dt.float8e4, tag="xp")
                    nc.sync.dma_start(
                        xp[:], xp_t.ap()[m].rearrange("p (t c) -> p t c", c=P))
                ps1 = pspool.tile([P, JC], dt.float32, tag="ps1")
                for k in range(0, KT, 2):
                    nc.tensor.matmul(
                        ps1[:],
                        xp[:, k:k + 2, :],
                        w1_sb[:, k:k + 2, :],
                        start=(k == 0),
                        stop=(k == KT - 2),
                        perf_mode=DR,
                    )
                # PSUM -> SBUF: H = (64H)/64, cast to fp8 (rhs of GEMM 2)
                nc.scalar.mul(h_sb[:, m, :], ps1[:], 1.0 / WSCALE)


            # ---- GEMM 2 (fp8 DoubleRow): 64*O1 = (64*A) @ H + 64*b1 ;
            #      x1 = sigmoid(O1) ; partial2[i] = sum_j x1[i,j] * W2[Cc_j] ----
            p2a_dram = drampool.tile([P, MA], dt.float32)
            p2b_dram = drampool.tile([P, MB], dt.float32)
            g2a_dram = drampool.tile([NCORES * P, MA], dt.float32, addr_space="Shared")
            g2b_dram = drampool.tile([NCORES * P, MB], dt.float32, addr_space="Shared")

            ATCH = (KT * JC) // 4
            for m in range(MT):
                app = apool.tile([P, KT, P], dt.float8e4, tag="app")
                nc.sync.dma_start(
                    app[:], ap_t.ap()[m].rearrange("p (t c) -> p t c", c=P))
                if 1 <= m <= 4:
                    # final-stage rhs: 4 x ~1MB chunks slipped between the
                    # early ap panels (small, absorbable bubbles on the queue)
                    a0 = (m - 1) * ATCH
                    nc.sync.dma_start(atrc_sb[:, a0:a0 + ATCH],
                                      atrc_t.ap()[:, a0:a0 + ATCH])
                ps2 = pspool.tile([P, JC], dt.float32, tag="ps2", bufs=3)
                for k in range(0, KT, 2):
                    nc.tensor.matmul(
                        ps2[:],
                        app[:, k:k + 2, :],
                        h_sb[:, k:k + 2, :],
                        start=(k == 0),
                        stop=False,
                        perf_mode=DR,
                    )
                # + 64*b1 as a bf16 rank-1 update: ones[1,128].T @ (64*b1)[1,512]
                nc.tensor.matmul(ps2[:], ones_sb[:], b1_sb[:], start=False, stop=True)
                x1t = x1pool.tile([P, JC], dt.float32, tag="x1t")
                # sigmoid((64*O1 + 64*b1) / 64)
                nc.scalar.activation(x1t[:], ps2[:], AF.Sigmoid, scale=1.0 / WSCALE)
                nc.vector.tensor_tensor(out=tts_sb[:], in0=x1t[:], in1=w2_sb[:],
                                        op=mybir.AluOpType.mult)
                nc.vector.tensor_reduce(out=p2_sb[:, m:m + 1], in_=tts_sb[:],
                                        axis=mybir.AxisListType.X,
                                        op=mybir.AluOpType.add)
                if m == MSPLIT - 1:
                    # chunk A: AllGather (half the ring steps of AllReduce) +
                    # on-core DVE sum; overlaps the rest of GEMM2
                    nc.gpsimd.dma_start(p2a_dram[:], p2_sb[:, 0:MA])
                    nc.gpsimd.collective_compute(
                        "AllGather", mybir.AluOpType.bypass,
                        replica_groups=[list(range(NCORES))],
                        ins=[p2a_dram.opt()], outs=[g2a_dram.opt()])
                    nc.gpsimd.dma_start(
                        ga_sb[:], g2a_dram.rearrange("(r p) m -> p r m", p=P))
                    nc.vector.tensor_tensor(out=ga_sb[:, 0, :], in0=ga_sb[:, 0, :],
                                            in1=ga_sb[:, 1, :],
                                            op=mybir.AluOpType.add)
                    nc.vector.tensor_tensor(out=ga_sb[:, 2, :], in0=ga_sb[:, 2, :],
                                            in1=ga_sb[:, 3, :],
                                            op=mybir.AluOpType.add)
                    nc.vector.tensor_tensor(out=ga_sb[:, 4, :], in0=ga_sb[:, 4, :],
                                            in1=ga_sb[:, 5, :],
                                            op=mybir.AluOpType.add)
                    nc.vector.tensor_tensor(out=ga_sb[:, 6, :], in0=ga_sb[:, 6, :],
                                            in1=ga_sb[:, 7, :],
                                            op=mybir.AluOpType.add)
                    nc.vector.tensor_tensor(out=ga_sb[:, 0, :], in0=ga_sb[:, 0, :],
                                            in1=ga_sb[:, 2, :],
                                            op=mybir.AluOpType.add)
                    nc.vector.tensor_tensor(out=ga_sb[:, 4, :], in0=ga_sb[:, 4, :],
                                            in1=ga_sb[:, 6, :],
                                            op=mybir.AluOpType.add)
                    nc.vector.tensor_tensor(out=h2fa_sb[:], in0=ga_sb[:, 0, :],
                                            in1=ga_sb[:, 4, :],
                                            op=mybir.AluOpType.add)
                    nc.vector.tensor_copy(out=h2ba_sb[:], in_=h2fa_sb[:])

            nc.gpsimd.dma_start(p2b_dram[:], p2_sb[:, MA:MT])
            nc.gpsimd.collective_compute(
                "AllGather", mybir.AluOpType.bypass,
                replica_groups=[list(range(NCORES))],
                ins=[p2b_dram.opt()], outs=[g2b_dram.opt()])
            nc.gpsimd.dma_start(
                gb_sb[:], g2b_dram.rearrange("(r p) m -> p r m", p=P))
            nc.vector.tensor_tensor(out=gb_sb[:, 0, :], in0=gb_sb[:, 0, :],
                                    in1=gb_sb[:, 1, :], op=mybir.AluOpType.add)
            nc.vector.tensor_tensor(out=gb_sb[:, 2, :], in0=gb_sb[:, 2, :],
                                    in1=gb_sb[:, 3, :], op=mybir.AluOpType.add)
            nc.vector.tensor_tensor(out=gb_sb[:, 4, :], in0=gb_sb[:, 4, :],
                                    in1=gb_sb[:, 5, :], op=mybir.AluOpType.add)
            nc.vector.tensor_tensor(out=gb_sb[:, 6, :], in0=gb_sb[:, 6, :],
                                    in1=gb_sb[:, 7, :], op=mybir.AluOpType.add)
            nc.vector.tensor_tensor(out=gb_sb[:, 0, :], in0=gb_sb[:, 0, :],
                                    in1=gb_sb[:, 2, :], op=mybir.AluOpType.add)
            nc.vector.tensor_tensor(out=gb_sb[:, 4, :], in0=gb_sb[:, 4, :],
                                    in1=gb_sb[:, 6, :], op=mybir.AluOpType.add)
            nc.vector.tensor_tensor(out=h2fb_sb[:], in0=gb_sb[:, 0, :],
                                    in1=gb_sb[:, 4, :], op=mybir.AluOpType.add)
            nc.vector.tensor_copy(out=h2bb_sb[:], in_=h2fb_sb[:])

            # ---- final (bf16): out[Rc]^T = sigmoid(h2^T @ AT[:, Rc] + b2) ----
            # k < MA uses chunk A (available while chunk B's AllReduce runs)
            ps3 = pspool.tile([1, JC], dt.float32, tag="ps3", bufs=1)
            for k in range(KT):
                lhs = h2ba_sb[:, k:k + 1] if k < MA else h2bb_sb[:, k - MA:k - MA + 1]
                nc.tensor.matmul(
                    ps3[:],
                    lhs,
                    atrc_sb[:, k * JC:(k + 1) * JC],
                    start=(k == 0),
                    stop=(k == KT - 1),
                )
            nc.scalar.activation(o_sb[:], ps3[:], AF.Sigmoid, bias=b2_sb[:])
            nc.gpsimd.dma_start(outc.ap(), o_sb[:])

    nc.compile()
    nc.m = get_hw_module(nc.m)
    return nc


def _host_preprocess(x, edge_index, W1, b1, W2, b2):
    """Build dense AT + pre-tiled fp8/bf16 operands; returns per-core in_maps."""
    edge_index = np.asarray(edge_index)
    src = edge_index[0].astype(np.int64)
    dst = edge_index[1].astype(np.int64)
    deg = np.bincount(dst, minlength=N).astype(np.float64) + 1.0
    dinv = 1.0 / np.sqrt(deg)
    vals = dinv[src] * dinv[dst]
    # AT[s, d] = A[d, s] (accumulates duplicate edges, like segment_sum)
    AT = np.bincount(src * N + dst, weights=vals, minlength=N * N)
    AT = AT.reshape(N, N)
    idx = np.arange(N)
    AT[idx, idx] += dinv * dinv
    AT32 = AT.astype(np.float32)

    x32 = np.asarray(x, dtype=np.float32)
    W1_32 = np.asarray(W1, dtype=np.float32)
    b1_32 = np.asarray(b1, dtype=np.float32)
    W2_32 = np.asarray(W2, dtype=np.float32).reshape(N)
    b2_32 = np.asarray(b2, dtype=np.float32).reshape(1)

    # xp_t[m, p, t*128+c] = x[m*128+c, t*128+p]
    xp_t = np.ascontiguousarray(
        x32.reshape(MT, P, KT, P).transpose(0, 3, 2, 1).reshape(MT, P, N)
    ).astype(_FP8)
    # ap_t[m, p, t*128+c] = 64*AT[t*128+p, m*128+c]
    ap_t = np.ascontiguousarray(
        (AT32 * np.float32(WSCALE)).reshape(KT, P, MT, P)
        .transpose(2, 1, 0, 3).reshape(MT, P, N)
    ).astype(_FP8)

    AT_b = AT32.astype(_BF16)
    W1_s = (W1_32 * np.float32(WSCALE)).astype(_FP8)

    in_maps = []
    for c in range(NCORES):
        cols = slice(c * JC, (c + 1) * JC)
        w1_t = np.ascontiguousarray(
            W1_s[:, cols].reshape(KT, P, JC).transpose(1, 0, 2).reshape(P, KT * JC)
        )
        atrc_t = np.ascontiguousarray(
            AT_b[:, cols].reshape(KT, P, JC).transpose(1, 0, 2).reshape(P, KT * JC)
        )
        in_maps.append({
            "xp_t": xp_t,
            "ap_t": ap_t,
            "w1_t": w1_t,
            "atrc_t": atrc_t,
            "b1c": (b1_32[cols] * np.float32(WSCALE)).reshape(1, JC).astype(_BF16),
            "w2b": np.ascontiguousarray(
                np.broadcast_to(W2_32[cols][None, :], (P, JC))
            ).astype(np.float32),
            "b2v": b2_32.reshape(1, 1).astype(np.float32),
        })
    return in_maps


def kernel(x, edge_index, W1, b1, W2, b2, _trace=False, _premaps=None):
    from concourse import bass_utils

    if "nc" not in _CACHE:
        _CACHE["nc"] = _build_bass_program()
    nc = _CACHE["nc"]

    in_maps = _premaps if _premaps is not None else _host_preprocess(
        x, edge_index, W1, b1, W2, b2)

    res = bass_utils.run_bass_kernel_spmd(
        nc, in_maps, core_ids=list(range(NCORES)), trace=_trace,
    )
    out = np.concatenate(
        [np.asarray(res.results[c]["outc"]).reshape(JC) for c in range(NCORES)]
    ).reshape(N, 1).astype(np.float32)
    if _trace:
        _CACHE["last_result"] = res
    return out


# revision 14
# speedup vs baseline: 1.0800x; 1.0369x over previous
"""Trainium2 Bass kernel for a 2-layer GCN discriminator (GCNConv -> sigmoid) x2.

Strategy
--------
With N=4096 nodes and E=262144 edges (avg degree 64), the gather/segment-sum
message passing is densified: the normalized adjacency
A[d, s] = sum_{edges (s,d)} dinv[s]*dinv[d]  (+ dinv[i]^2 self loops)
is built on the host as a dense 4096x4096 matrix.  The whole network is then

    x1  = sigmoid(A @ (x @ W1) + b1)
    out = sigmoid(A @ (x1 @ W2) + b2)

i.e. two dense 4096^3 GEMMs plus small epilogues -- ideal for the PE array.

Sharding over 8 cores: layer-1 is column-sharded (each core computes 512
columns of H = x@W1 and of x1), the tiny layer-2 contraction
h2 = x1 @ W2 is computed per-core on the local 512 columns and summed across
cores with a 16KB AllGather + on-core tree sum (split in two chunks so the
first mostly overlaps GEMM2), and the final out rows are row-sharded
(512 rows per core).

The two big GEMMs run in fp8-e4m3 with DoubleRow perf mode (2 contraction
rows per PE cell per cycle).  W1 and A are pre-scaled by 64 on the host so
their entries sit in e4m3's normal range; the scale is removed for free in
the activation epilogues (ACT computes func(in*scale + bias)).  PSUM
accumulation is fp32 throughout; the small final GEMM stays bf16.
"""

import numpy as np
import ml_dtypes

N = 4096
E = 262144
P = 128
NCORES = 8
JC = N // NCORES          # 512 feature-cols (layer1) / out-rows (final) per core
KT = N // P               # 32 contraction tiles
MT = N // P               # 32 output row tiles
MSPLIT = 24               # collective chunk A covers m-tiles [0, MSPLIT)
WSCALE = 64.0             # host pre-scale on W1 and A for fp8 range

_BF16 = ml_dtypes.bfloat16
_FP8 = ml_dtypes.float8_e4m3

_CACHE = {}


def _build_bass_program(with_b1=True):
    """Build + compile the SPMD Bass program (identical on all 8 cores)."""
    import concourse.bass as bass  # noqa: F401
    import concourse.bacc as bacc
    import concourse.tile as tile
    import concourse.mybir as mybir
    from concourse.bass_interp import get_hw_module

    dt = mybir.dt
    AF = mybir.ActivationFunctionType
    DR = mybir.MatmulPerfMode.DoubleRow

    nc = bacc.Bacc("TRN2", target_bir_lowering=False, debug=False,
                   num_devices=NCORES)

    # ---- kernel I/O (per-core) ----
    # xp_t[m, p, t*128+c] = x[m*128+c, t*128+p]      (pre-tiled lhsT panels)
    xp_t = nc.dram_tensor("xp_t", [MT, P, N], dt.float8e4, kind="ExternalInput")
    # ap_t[m, p, t*128+c] = 64*AT[t*128+p, m*128+c]  (AT[s,d] = A[d,s])
    ap_t = nc.dram_tensor("ap_t", [MT, P, N], dt.float8e4, kind="ExternalInput")
    # w1_t[p, t*512+j] = 64*W1[t*128+p, c*JC+j]
    w1_t = nc.dram_tensor("w1_t", [P, KT * JC], dt.float8e4, kind="ExternalInput")
    # atrc_t[p, t*512+j] = AT[t*128+p, c*JC+j]       (final-stage rhs panels)
    atrc_t = nc.dram_tensor("atrc_t", [P, KT * JC], dt.bfloat16, kind="ExternalInput")
    b1c = (nc.dram_tensor("b1c", [1, JC], dt.bfloat16, kind="ExternalInput")
           if with_b1 else None)  # 64*b1
    w2b = nc.dram_tensor("w2b", [P, JC], dt.float32, kind="ExternalInput")
    b2v = nc.dram_tensor("b2v", [1, 1], dt.float32, kind="ExternalInput")
    outc = nc.dram_tensor("outc", [1, JC], dt.float32, kind="ExternalOutput")

    MA = MSPLIT
    MB = MT - MSPLIT

    with tile.TileContext(nc) as tc:
        with tc.tile_pool(name="const", bufs=1) as const, \
             tc.tile_pool(name="xpool", bufs=4) as xpool, \
             tc.tile_pool(name="apool", bufs=4) as apool, \
             tc.tile_pool(name="x1pool", bufs=3) as x1pool, \
             tc.tile_pool(name="pspool", bufs=2, space="PSUM") as pspool, \
             tc.tile_pool(name="drampool", bufs=1, space="DRAM") as drampool:

            # ---- resident SBUF tensors ----
            w1_sb = const.tile([P, KT, JC], dt.float8e4)
            atrc_sb = const.tile([P, KT * JC], dt.bfloat16)
            h_sb = const.tile([P, MT, JC], dt.float8e4)
            w2_sb = const.tile([P, JC], dt.float32)
            b1_sb = const.tile([1, JC], dt.bfloat16) if with_b1 else None
            b2_sb = const.tile([1, 1], dt.float32)
            ones_sb = const.tile([1, P], dt.bfloat16) if with_b1 else None
            p2_sb = const.tile([P, MT], dt.float32)
            h2fa_sb = const.tile([P, MSPLIT], dt.float32)
            h2ba_sb = const.tile([P, MSPLIT], dt.bfloat16)
            h2fb_sb = const.tile([P, MT - MSPLIT], dt.float32)
            h2bb_sb = const.tile([P, MT - MSPLIT], dt.bfloat16)
            ga_sb = const.tile([P, NCORES, MSPLIT], dt.float32)
            gb_sb = const.tile([P, NCORES, MT - MSPLIT], dt.float32)
            o_sb = const.tile([1, JC], dt.float32)
            tts_sb = const.tile([P, JC], dt.float32)

            # First matmul needs xp[0] + the first w1 chunk; keep those at the
            # head of the DMA queue (each dma_start has ~1us first-byte cost,
            # so w1 goes in 4 chunks, not 32 small ones).
            xp0 = xpool.tile([P, KT, P], dt.float8e4, tag="xp")
            nc.sync.dma_start(
                w1_sb[:, 0:2, :],
                w1_t.ap()[:, 0:2 * JC].rearrange("p (t j) -> p t j", j=JC))
            nc.sync.dma_start(xp0[:], xp_t.ap()[0].rearrange("p (t c) -> p t c", c=P))
            W1CH = 10
            for k in range(2, KT, W1CH):
                ke = min(k + W1CH, KT)
                nc.sync.dma_start(
                    w1_sb[:, k:ke, :],
                    w1_t.ap()[:, k * JC:ke * JC].rearrange(
                        "p (t j) -> p t j", j=JC))
            nc.gpsimd.dma_start(w2_sb[:], w2b.ap())
            if with_b1:
                nc.gpsimd.dma_start(b1_sb[:], b1c.ap())
            nc.gpsimd.dma_start(b2_sb[:], b2v.ap())
            if with_b1:
                nc.vector.memset(ones_sb[:], 1.0)

            # ---- GEMM 1 (fp8 DoubleRow): 64*H[:, Cc] = x @ (64*W1[:, Cc]) ----
            for m in range(MT):
                if m == 0:
                    xp = xp0
                else:
                    xp = xpool.tile([P, KT, P], dt.float8e4, tag="xp")
                    nc.sync.dma_start(
                        xp[:], xp_t.ap()[m].rearrange("p (t c) -> p t c", c=P))
                ps1 = pspool.tile([P, JC], dt.float32, tag="ps1")
                for k in range(0, KT, 2):
                    nc.tensor.matmul(
                        ps1[:],
                        xp[:, k:k + 2, :],
                        w1_sb[:, k:k + 2, :],
                        start=(k == 0),
                        stop=(k == KT - 2),
                        perf_mode=DR,
                    )
                # PSUM -> SBUF: H = (64H)/64, cast to fp8 (rhs of GEMM 2)
                nc.scalar.mul(h_sb[:, m, :], ps1[:], 1.0 / WSCALE)


            # ---- GEMM 2 (fp8 DoubleRow): 64*O1 = (64*A) @ H + 64*b1 ;
            #      x1 = sigmoid(O1) ; partial2[i] = sum_j x1[i,j] * W2[Cc_j] ----
            p2a_dram = drampool.tile([P, MA], dt.float32)
            p2b_dram = drampool.tile([P, MB], dt.float32)
            g2a_dram = drampool.tile([NCORES * P, MA], dt.float32, addr_space="Shared")
            g2b_dram = drampool.tile([NCORES * P, MB], dt.float32, addr_space="Shared")

            ATCH = (KT * JC) // 4
            for m in range(MT):
                app = apool.tile([P, KT, P], dt.float8e4, tag="app")
                nc.sync.dma_start(
                    app[:], ap_t.ap()[m].rearrange("p (t c) -> p t c", c=P))
                if 1 <= m <= 4:
                    # final-stage rhs: 4 x ~1MB chunks slipped between the
                    # early ap panels (small, absorbable bubbles on the queue)
                    a0 = (m - 1) * ATCH
                    nc.sync.dma_start(atrc_sb[:, a0:a0 + ATCH],
                                      atrc_t.ap()[:, a0:a0 + ATCH])
                ps2 = pspool.tile([P, JC], dt.float32, tag="ps2", bufs=3)
                for k in range(0, KT, 2):
                    nc.tensor.matmul(
                        ps2[:],
                        app[:, k:k + 2, :],
                        h_sb[:, k:k + 2, :],
                        start=(k == 0),
                        stop=(not with_b1 and k == KT - 2),
                        perf_mode=DR,
                    )
                if with_b1:
                    # + 64*b1 as a bf16 rank-1 update: ones.T @ (64*b1)
                    nc.tensor.matmul(ps2[:], ones_sb[:], b1_sb[:],
                                     start=False, stop=True)
                x1t = x1pool.tile([P, JC], dt.float32, tag="x1t")
                # sigmoid((64*O1 + 64*b1) / 64)
                nc.scalar.activation(x1t[:], ps2[:], AF.Sigmoid, scale=1.0 / WSCALE)
                nc.vector.tensor_tensor(out=tts_sb[:], in0=x1t[:], in1=w2_sb[:],
                                        op=mybir.AluOpType.mult)
                nc.vector.tensor_reduce(out=p2_sb[:, m:m + 1], in_=tts_sb[:],
                                        axis=mybir.AxisListType.X,
                                        op=mybir.AluOpType.add)
                if m == MSPLIT - 1:
                    # chunk A: AllGather (half the ring steps of AllReduce) +
                    # on-core DVE sum; overlaps the rest of GEMM2
                    nc.gpsimd.dma_start(p2a_dram[:], p2_sb[:, 0:MA])
                    nc.gpsimd.collective_compute(
                        "AllGather", mybir.AluOpType.bypass,
                        replica_groups=[list(range(NCORES))],
                        ins=[p2a_dram.opt()], outs=[g2a_dram.opt()])
                    nc.gpsimd.dma_start(
                        ga_sb[:], g2a_dram.rearrange("(r p) m -> p r m", p=P))
                    nc.vector.tensor_tensor(out=ga_sb[:, 0, :], in0=ga_sb[:, 0, :],
                                            in1=ga_sb[:, 1, :],
                                            op=mybir.AluOpType.add)
                    nc.vector.tensor_tensor(out=ga_sb[:, 2, :], in0=ga_sb[:, 2, :],
                                            in1=ga_sb[:, 3, :],
                                            op=mybir.AluOpType.add)
                    nc.vector.tensor_tensor(out=ga_sb[:, 4, :], in0=ga_sb[:, 4, :],
                                            in1=ga_sb[:, 5, :],
                                            op=mybir.AluOpType.add)
                    nc.vector.tensor_tensor(out=ga_sb[:, 6, :], in0=ga_sb[:, 6, :],
                                            in1=ga_sb[:, 7, :],
                                            op=mybir.AluOpType.add)
                    nc.vector.tensor_tensor(out=ga_sb[:, 0, :], in0=ga_sb[:, 0, :],
                                            in1=ga_sb[:, 2, :],
                                            op=mybir.AluOpType.add)
                    nc.vector.tensor_tensor(out=ga_sb[:, 4, :], in0=ga_sb[:, 4, :],
                                            in1=ga_sb[:, 6, :],
                                            op=mybir.AluOpType.add)
                    nc.vector.tensor_tensor(out=h2fa_sb[:], in0=ga_sb[:, 0, :],
                                            in1=ga_sb[:, 4, :],
                                            op=mybir.AluOpType.add)
                    nc.vector.tensor_copy(out=h2ba_sb[:], in_=h2fa_sb[:])

            nc.gpsimd.dma_start(p2b_dram[:], p2_sb[:, MA:MT])
            nc.gpsimd.collective_compute(
                "AllGather", mybir.AluOpType.bypass,
                replica_groups=[list(range(NCORES))],
                ins=[p2b_dram.opt()], outs=[g2b_dram.opt()])
            nc.gpsimd.dma_start(
                gb_sb[:], g2b_dram.rearrange("(r p) m -> p r m", p=P))
            nc.vector.tensor_tensor(out=gb_sb[:, 0, :], in0=gb_sb[:, 0, :],
                                    in1=gb_sb[:, 1, :], op=mybir.AluOpType.add)
            nc.vector.tensor_tensor(out=gb_sb[:, 2, :], in0=gb_sb[:, 2, :],
                                    in1=gb_sb[:, 3, :], op=mybir.AluOpType.add)
            nc.vector.tensor_tensor(out=gb_sb[:, 4, :], in0=gb_sb[:, 4, :],
                                    in1=gb_sb[:, 5, :], op=mybir.AluOpType.add)
            nc.vector.tensor_tensor(out=gb_sb[:, 6, :], in0=gb_sb[:, 6, :],
                                    in1=gb_sb[:, 7, :], op=mybir.AluOpType.add)
            nc.vector.tensor_tensor(out=gb_sb[:, 0, :], in0=gb_sb[:, 0, :],
                                    in1=gb_sb[:, 2, :], op=mybir.AluOpType.add)
            nc.vector.tensor_tensor(out=gb_sb[:, 4, :], in0=gb_sb[:, 4, :],
                                    in1=gb_sb[:, 6, :], op=mybir.AluOpType.add)
            nc.vector.tensor_tensor(out=h2fb_sb[:], in0=gb_sb[:, 0, :],
                                    in1=gb_sb[:, 4, :], op=mybir.AluOpType.add)
            nc.vector.tensor_copy(out=h2bb_sb[:], in_=h2fb_sb[:])

            # ---- final (bf16): out[Rc]^T = sigmoid(h2^T @ AT[:, Rc] + b2) ----
            # k < MA uses chunk A (available while chunk B's AllReduce runs)
            ps3 = pspool.tile([1, JC], dt.float32, tag="ps3", bufs=1)
            for k in range(KT):
                lhs = h2ba_sb[:, k:k + 1] if k < MA else h2bb_sb[:, k - MA:k - MA + 1]
                nc.tensor.matmul(
                    ps3[:],
                    lhs,
                    atrc_sb[:, k * JC:(k + 1) * JC],
                    start=(k == 0),
                    stop=(k == KT - 1),
                )
            nc.scalar.activation(o_sb[:], ps3[:], AF.Sigmoid, bias=b2_sb[:])
            nc.gpsimd.dma_start(outc.ap(), o_sb[:])

    nc.compile()
    nc.m = get_hw_module(nc.m)
    return nc


def _host_preprocess(x, edge_index, W1, b1, W2, b2):
    """Build dense AT + pre-tiled fp8/bf16 operands; returns per-core in_maps."""
    edge_index = np.asarray(edge_index)
    src = edge_index[0].astype(np.int64)
    dst = edge_index[1].astype(np.int64)
    deg = np.bincount(dst, minlength=N).astype(np.float64) + 1.0
    dinv = 1.0 / np.sqrt(deg)
    vals = dinv[src] * dinv[dst]
    # AT[s, d] = A[d, s] (accumulates duplicate edges, like segment_sum)
    AT = np.bincount(src * N + dst, weights=vals, minlength=N * N)
    AT = AT.reshape(N, N)
    idx = np.arange(N)
    AT[idx, idx] += dinv * dinv
    AT32 = AT.astype(np.float32)

    x32 = np.asarray(x, dtype=np.float32)
    W1_32 = np.asarray(W1, dtype=np.float32)
    b1_32 = np.asarray(b1, dtype=np.float32)
    W2_32 = np.asarray(W2, dtype=np.float32).reshape(N)
    b2_32 = np.asarray(b2, dtype=np.float32).reshape(1)

    # xp_t[m, p, t*128+c] = x[m*128+c, t*128+p]
    xp_t = np.ascontiguousarray(
        x32.reshape(MT, P, KT, P).transpose(0, 3, 2, 1).reshape(MT, P, N)
    ).astype(_FP8)
    # ap_t[m, p, t*128+c] = 64*AT[t*128+p, m*128+c]
    ap_t = np.ascontiguousarray(
        (AT32 * np.float32(WSCALE)).reshape(KT, P, MT, P)
        .transpose(2, 1, 0, 3).reshape(MT, P, N)
    ).astype(_FP8)

    AT_b = AT32.astype(_BF16)
    W1_s = (W1_32 * np.float32(WSCALE)).astype(_FP8)

    in_maps = []
    for c in range(NCORES):
        cols = slice(c * JC, (c + 1) * JC)
        w1_t = np.ascontiguousarray(
            W1_s[:, cols].reshape(KT, P, JC).transpose(1, 0, 2).reshape(P, KT * JC)
        )
        atrc_t = np.ascontiguousarray(
            AT_b[:, cols].reshape(KT, P, JC).transpose(1, 0, 2).reshape(P, KT * JC)
        )
        in_maps.append({
            "xp_t": xp_t,
            "ap_t": ap_t,
            "w1_t": w1_t,
            "atrc_t": atrc_t,
            "b1c": (b1_32[cols] * np.float32(WSCALE)).reshape(1, JC).astype(_BF16),
            "w2b": np.ascontiguousarray(
                np.broadcast_to(W2_32[cols][None, :], (P, JC))
            ).astype(np.float32),
            "b2v": b2_32.reshape(1, 1).astype(np.float32),
        })
    return in_maps


def kernel(x, edge_index, W1, b1, W2, b2, _trace=False, _premaps=None):
    from concourse import bass_utils

    with_b1 = bool(np.any(np.asarray(b1)))
    key = f"nc_b1={with_b1}"
    if key not in _CACHE:
        _CACHE[key] = _build_bass_program(with_b1=with_b1)
    nc = _CACHE[key]

    in_maps = _premaps if _premaps is not None else _host_preprocess(
        x, edge_index, W1, b1, W2, b2)
    if not with_b1:
        in_maps = [{k: v for k, v in m.items() if k != "b1c"} for m in in_maps]

    res = bass_utils.run_bass_kernel_spmd(
        nc, in_maps, core_ids=list(range(NCORES)), trace=_trace,
    )
    out = np.concatenate(
        [np.asarray(res.results[c]["outc"]).reshape(JC) for c in range(NCORES)]
    ).reshape(N, 1).astype(np.float32)
    if _trace:
        _CACHE["last_result"] = res
    return out
